# revision 1
# baseline (speedup 1.0000x reference)
"""DIF (dual-softmax) attention layer on 8 Trainium2 NeuronCores.

Sharding: core = (batch b, head-stack s), b in 0..3, s in 0..1.
Each core computes, for its batch and its 4 heads, the full dual-softmax
attention over all T rows, producing a partial output projection (sum over
its 4 heads; bias folded into stack 0). Host sums the two stack partials.

On-chip layout ("S^T" / flash style), q-tile=512, k-tile=128:
  - Q^T, K^T per branch: [128 (4h x 32d), T] bf16 in SBUF; attention scale
    * log2(e) folded into the Q projection weights (exp runs in base 2).
  - scores: per (k-tile, head-pair) event, 2 row-packed matmuls
    (tile_position=(32h,0)) into one [128, 1024] PSUM tile; the two heads
    land in different banks (concurrent row-tiled matmuls sharing a bank
    are device-fatal).
  - exp: one ScalarE activation per event, [128, 1024], scale=ln2.
  - P@V and denominators: col-packed matmuls (tile_position=(0,32h)); O^T
    accumulates in one PSUM bank, denominators (ones-matmul, M=32 so each
    head's denom is replicated across its 32 partitions) in another.
  - diagonal k-tiles: only the valid q-range is computed (qlo=128*di) and
    the 128-col triangle gets a -1e30 bias add before exp.
  - normalize: reciprocal_approx_fast(denom) * O^T -> bf16.
  - output projection: lhsT = normalized O^T [128,128] slices, rhs = Wo^T
    slices pre-scaled by alpha / (1-alpha); both branches accumulate into
    one PSUM bank; broadcast-bias add; DMA out.
"""

import numpy as np
import ml_dtypes

import concourse.bass as bass
import concourse.tile as tile
from concourse import bacc, mybir, dve_ops
from concourse.dve_spec import (Spec, Src0, C0, C1, C2, C3, One, Idx,
                                lower, _spill_c3_to_src1, _has_src1 as has_src1)
from concourse.dve_uop import DveOpSpec
from concourse.bass_utils import run_bass_kernel_spmd

B, D, H, HD = 4, 256, 8, 32
HPS = 4  # heads per stack (per core)
LOG2E = 1.4426950408889634
LN2 = 0.6931471805599453
QT = 512  # q-tile width
KT = 128  # k-tile width
NEG = -1.0e30
KEXP = 32  # exp2 split factor: exp2(y) = p(y/KEXP)^KEXP on the DVE path

# minimax coefficients for p(z) = 1 + z(a + z(b + z(c + d z))) ~ 2^z, |z|<=0.5
PA, PB, PC, PD = 0.693128038, 0.24023678, 0.055870371, 0.009590248

BF16 = mybir.dt.bfloat16
F32 = mybir.dt.float32
AF = mybir.ActivationFunctionType
OP = mybir.AluOpType

_prog_cache: dict = {}


def _register_dve_op(name, spec, subdim=False):
    """Register a custom DVE op at import time, self-pinning its uops sha."""
    for op in dve_ops.OPS:
        if op.name == name:
            return op
    row = dve_ops._CUSTOM_DVE_ROW_BASE + len(dve_ops.OPS)
    shas = {}
    for ver in ("v3", "v4"):
        s = DveOpSpec(name=name, opcode=row, uops=lower(spec, ver=ver),
                      rd1_en=has_src1(spec))
        shas[ver] = s.sha(ver)
    op = dve_ops.DveOp(name, spec, subdim=subdim, uops_sha=shas)
    dve_ops.OPS.append(op)
    dve_ops._SUB_OPCODE_FOR_NAME[name] = row
    dve_ops.CUSTOM_DVE_SPECS[name] = spec
    return op


def _make_exp_ops():
    z = Src0
    poly = One + z * (C0 + z * (C1 + z * (C2 + C3 * z)))
    k1 = _register_dve_op("ANT_EXP2_POLY", Spec(
        body=_spill_c3_to_src1(poly),
        reference=lambda in0, in1, s0, s1, imm2:
            1.0 + in0 * (s0 + in0 * (s1 + in0 * (
                imm2 + np.reshape(in1, (-1,) + (1,) * (in0.ndim - 1)) * in0))),
    ))
    p = Src0
    for _ in range(5):
        p = p * p
    k2m = _register_dve_op("ANT_EXP2_SQ5M", Spec(
        body=p * (Idx >= C0),
        reference=lambda in0, in1, s0, s1, imm2:
            (in0.astype(np.float32) ** 32)
            * (np.arange(in0.shape[-1], dtype=np.float32)
               >= np.reshape(s0, (-1,) + (1,) * (in0.ndim - 1))),
    ))
    p = Src0
    for _ in range(5):
        p = p * p
    k2 = _register_dve_op("ANT_EXP2_SQ5", Spec(
        body=p,
        reference=lambda in0, in1, s0, s1, imm2: in0.astype(np.float32) ** 32,
    ))
    return k1, k2m, k2


EXP2P, EXP2SQ5M, EXP2SQ5 = _make_exp_ops()


def _build_program(T, causal=True):
    nc = bacc.Bacc("TRN2", target_bir_lowering=False, debug=False)

    xc = nc.dram_tensor("xc", [2, 128, T], BF16, kind="ExternalInput")
    xk = nc.dram_tensor("xk", [2, 128, T], BF16, kind="ExternalInput")
    wqc = nc.dram_tensor("wqc", [2, 128, 128], BF16, kind="ExternalInput")
    wkc = nc.dram_tensor("wkc", [2, 128, 128], BF16, kind="ExternalInput")
    wqk = nc.dram_tensor("wqk", [2, 128, 128], BF16, kind="ExternalInput")
    wkk = nc.dram_tensor("wkk", [2, 128, 128], BF16, kind="ExternalInput")
    wv = nc.dram_tensor("wv", [2, 128, 128], BF16, kind="ExternalInput")
    woc = nc.dram_tensor("woc", [128, 256], BF16, kind="ExternalInput")
    wok = nc.dram_tensor("wok", [128, 256], BF16, kind="ExternalInput")
    bo_b = nc.dram_tensor("bo_b", [128, 256], F32, kind="ExternalInput")
    pio = nc.dram_tensor("pio", [128, 1], F32, kind="ExternalInput")
    mzro = nc.dram_tensor("mzro", [128, 2 * KT], BF16, kind="ExternalInput")
    y = nc.dram_tensor("y", [T, 256], F32, kind="ExternalOutput")

    NQT = T // QT
    NTT = T // KT

    with tile.TileContext(nc) as tc:
        with (
            tc.tile_pool(name="xin", bufs=1) as xin,
            tc.tile_pool(name="wts", bufs=1) as wts,
            tc.tile_pool(name="proj", bufs=1) as proj,
            tc.tile_pool(name="exps", bufs=4) as exps,
            tc.tile_pool(name="ex1p", bufs=3) as ex1p,
            tc.tile_pool(name="onrm", bufs=4) as onrm,
            tc.tile_pool(name="recp", bufs=2) as recp,
            tc.tile_pool(name="yout", bufs=3) as yout,
            tc.tile_pool(name="ps_sc", bufs=2, space="PSUM") as ps_sc,
            tc.tile_pool(name="ps_pv", bufs=1, space="PSUM") as ps_pv,
            tc.tile_pool(name="ps_dn", bufs=1, space="PSUM") as ps_dn,
            tc.tile_pool(name="ps_mm", bufs=2, space="PSUM") as ps_mm,
        ):
            # ---- load inputs ----
            xc0 = xin.tile([128, T], BF16, tag="xc0")
            xc1 = xin.tile([128, T], BF16, tag="xc1")
            xk0 = xin.tile([128, T], BF16, tag="xk0")
            xk1 = xin.tile([128, T], BF16, tag="xk1")
            nch = 2 if T >= 1024 else 1
            for ch in range(nch):
                sl = bass.ts(ch, T // nch)
                nc.sync.dma_start(xc0[:, sl], xc[0][:, sl])
                nc.sync.dma_start(xc1[:, sl], xc[1][:, sl])
                nc.sync.dma_start(xk0[:, sl], xk[0][:, sl])
                nc.sync.dma_start(xk1[:, sl], xk[1][:, sl])

            # warm the ACT exp table while DMAs stream in
            warm = wts.tile([128, 1], F32, tag="warm")
            nc.vector.memset(warm[:], 0.0)
            nc.scalar.activation(warm[:], warm[:], AF.Exp, scale=1.0)

            w_sb = {}
            for nm, dram in [("wqc", wqc), ("wkc", wkc), ("wqk", wqk),
                             ("wkk", wkk), ("wv", wv)]:
                for j in range(2):
                    t = wts.tile([128, 128], BF16, tag=f"{nm}{j}")
                    nc.sync.dma_start(t[:], dram[j])
                    w_sb[(nm, j)] = t
            woc_sb = wts.tile([128, 256], BF16, tag="woc")
            wok_sb = wts.tile([128, 256], BF16, tag="wok")
            bo_sb = wts.tile([128, 256], F32, tag="bo")
            pio_sb = wts.tile([128, 1], F32, tag="pio")
            mzro_sb = wts.tile([128, 2 * KT], BF16, tag="mzro")
            nc.sync.dma_start(woc_sb[:], woc[:])
            nc.sync.dma_start(wok_sb[:], wok[:])
            nc.sync.dma_start(bo_sb[:], bo_b[:])
            nc.sync.dma_start(pio_sb[:], pio[:])
            nc.sync.dma_start(mzro_sb[:], mzro[:])
            ones_sb = wts.tile([128, 32], BF16, tag="ones")
            nc.vector.memset(ones_sb[:], 1.0)
            dco_sb = wts.tile([128, 1], F32, tag="dco")
            nc.vector.memset(dco_sb[:], PD)

            # ---- projections ----
            # Q^T/K^T: out[i, t] = sum_j W.T[j, i] * x^T[j, t]
            qkt = {}
            for nm, xs in [("wqc", (xc0, xc1)), ("wkc", (xc0, xc1)),
                           ("wqk", (xk0, xk1)), ("wkk", (xk0, xk1))]:
                dst = proj.tile([128, T], BF16, tag=f"p_{nm}")
                qkt[nm] = dst
                for nt in range(T // 512):
                    ps = ps_mm.tile([128, 512], F32, tag="mm")
                    sl = bass.ts(nt, 512)
                    nc.tensor.matmul(ps[:], w_sb[(nm, 0)][:], xs[0][:, sl],
                                     start=True, stop=False)
                    nc.tensor.matmul(ps[:], w_sb[(nm, 1)][:], xs[1][:, sl],
                                     start=False, stop=True)
                    if nt % 4 == 3:
                        nc.vector.tensor_copy(dst[:, sl], ps[:])
                    else:
                        nc.scalar.copy(dst[:, sl], ps[:])
            q_c, k_c, q_k, k_k = qkt["wqc"], qkt["wkc"], qkt["wqk"], qkt["wkk"]

            # V: out[t, i] = sum_j x^T[j, t] * Wv.T[j, i]; layout [128, NTT, 128]
            v_sb = proj.tile([128, NTT, 128], BF16, tag="p_v")
            for tt in range(NTT):
                ps = ps_mm.tile([128, 512], F32, tag="mm")
                sl = bass.ts(tt, 128)
                nc.tensor.matmul(ps[:, 0:128], xc0[:, sl], w_sb[("wv", 0)][:],
                                 start=True, stop=False)
                nc.tensor.matmul(ps[:, 0:128], xc1[:, sl], w_sb[("wv", 1)][:],
                                 start=False, stop=True)
                nc.scalar.copy(v_sb[:, tt, :], ps[:, 0:128])

            # ---- attention ----
            for qt in range(NQT):
                q0 = qt * QT
                nkt = (q0 + QT) // KT if causal else NTT
                on_tiles = {}
                # diagonal k-tiles are the last 4; interleave them among the
                # clean ones so the DVE and ACT exp streams overlap.
                nmask = 4 if causal else 0
                kts_clean = list(range(nkt - nmask))
                kts_mask = list(range(nkt - nmask, nkt))
                order = []
                if kts_clean:
                    stride = max(1, len(kts_clean) // 4)
                    mi = 0
                    for i, kt in enumerate(kts_clean):
                        order.append(kt)
                        if (i + 1) % stride == 0 and mi < nmask:
                            order.append(kts_mask[mi])
                            mi += 1
                    order += kts_mask[mi:]
                else:
                    order = kts_mask
                for br, (qsb, ksb) in [("c", (q_c, k_c)), ("k", (q_k, k_k))]:
                    pv = ps_pv.tile([128, 512], F32, tag="pv")
                    dn = ps_dn.tile([128, 512], F32, tag="dn")
                    pend = None
                    for kt in order:
                        k0 = kt * KT
                        di = kt - (nkt - 4) if causal else -1
                        qlo = 128 * di if di > 0 else 0
                        for hp in range(2):
                            # all-masked waves (qt==0) split events over both
                            # engines; otherwise masked -> DVE, clean -> ACT,
                            # plus a few deep-wave clean events to DVE for
                            # load balance.
                            use_dve = di >= 0 and (kts_clean or (kt + hp) % 2 == 0)
                            if causal:
                                use_dve_clean = (di < 0 and qt == 3 and kt == 2
                                                 and hp == 0)
                            else:
                                use_dve_clean = (2 * kt + hp) % 7 < 2
                            sp = ps_sc.tile([128, 2 * QT], F32, tag="sc")
                            for hl in range(2):
                                h = 2 * hp + hl
                                nc.tensor.matmul(
                                    sp[:, QT * hl + qlo: QT * (hl + 1)],
                                    ksb[32 * h:32 * h + 32, k0:k0 + KT],
                                    qsb[32 * h:32 * h + 32, q0 + qlo:q0 + QT],
                                    start=True, stop=True,
                                    tile_position=(32 * h, 0),
                                    skip_group_check=True,
                                )
                            ex = exps.tile([128, 2 * QT], BF16, tag="ex")
                            if use_dve_clean:
                                e1 = ex1p.tile([128, 2 * QT], F32, tag="e1")
                                nc.vector._custom_dve(
                                    EXP2P, out=e1[:], in0=sp[:],
                                    in1=dco_sb[:, 0:1], s0=PA, s1=PB, imm2=PC)
                                nc.vector._custom_dve(EXP2SQ5, out=ex[:], in0=e1[:])
                            elif use_dve:
                                # DVE exp: poly + 5 squarings, causal mask via
                                # the (Idx >= p) comparison in the last stage.
                                e1 = ex1p.tile([128, 2 * QT], F32, tag="e1")
                                spv = sp[:].rearrange("p (l q) -> p l q", l=2)
                                e1v = e1[:].rearrange("p (l q) -> p l q", l=2)
                                nc.vector._custom_dve(
                                    EXP2P, out=e1v[:, :, qlo:], in0=spv[:, :, qlo:],
                                    in1=dco_sb[:, 0:1], s0=PA, s1=PB, imm2=PC)
                                for hl in range(2):
                                    nc.vector._custom_dve(
                                        EXP2SQ5M,
                                        out=ex[:, QT * hl + qlo:QT * (hl + 1)],
                                        in0=e1[:, QT * hl + qlo:QT * (hl + 1)],
                                        s0=pio_sb[:, 0:1])
                            else:
                                if qlo:
                                    nc.scalar.activation(
                                        ex[:].rearrange("p (l q) -> p l q", l=2)[:, :, qlo:],
                                        sp[:].rearrange("p (l q) -> p l q", l=2)[:, :, qlo:],
                                        AF.Exp, scale=KEXP * LN2)
                                else:
                                    nc.scalar.activation(ex[:], sp[:], AF.Exp,
                                                         scale=KEXP * LN2)
                                if di >= 0:
                                    # zero the causal triangle post-exp
                                    exv = ex[:].rearrange(
                                        "p (l q) -> p l q", l=2)[:, :, qlo:qlo + KT]
                                    nc.vector.tensor_tensor(
                                        exv, exv,
                                        mzro_sb[:].rearrange("p (l q) -> p l q", l=2),
                                        OP.mult)
                            if pend is not None:
                                _pv_den(nc, pv, dn, v_sb, ones_sb, *pend)
                            pend = (ex, kt, hp, qlo, kt == order[0])
                    _pv_den(nc, pv, dn, v_sb, ones_sb, *pend)

                    rec = recp.tile([128, 512], F32, tag="rec")
                    nc.vector.reciprocal_approx_fast(rec[:], dn[:])
                    on = onrm.tile([128, 512], BF16, tag=f"on{br}")
                    nc.vector.tensor_tensor(on[:], pv[:], rec[:], OP.mult)
                    on_tiles[br] = on

                for m in range(QT // 128):
                    yp = ps_mm.tile([128, 512], F32, tag="mm")
                    nc.tensor.matmul(yp[:, 0:256],
                                     on_tiles["c"][:, bass.ts(m, 128)],
                                     woc_sb[:], start=True, stop=False)
                    nc.tensor.matmul(yp[:, 0:256],
                                     on_tiles["k"][:, bass.ts(m, 128)],
                                     wok_sb[:], start=False, stop=True)
                    ysb = yout.tile([128, 256], F32, tag="y")
                    nc.vector.tensor_tensor(ysb[:], yp[:, 0:256], bo_sb[:], OP.add)
                    nc.sync.dma_start(y[q0 + m * 128:q0 + (m + 1) * 128, :], ysb[:])

    nc.compile()
    return nc


def _pv_den(nc, pv, dn, v_sb, ones_sb, ex, kt, hp, qlo, first):
    # pv pair first, then dn pair: consecutive matmuls on distinct col
    # groups can run concurrently in the PE array.
    for hl in range(2):
        h = 2 * hp + hl
        rhs = ex[:, QT * hl + qlo:QT * (hl + 1)]
        nc.tensor.matmul(pv[32 * h:32 * h + 32, qlo:QT],
                         v_sb[:, kt, 32 * h:32 * h + 32], rhs,
                         start=first, stop=False,
                         tile_position=(0, 32 * h), skip_group_check=True)
    for hl in range(2):
        h = 2 * hp + hl
        rhs = ex[:, QT * hl + qlo:QT * (hl + 1)]
        nc.tensor.matmul(dn[32 * h:32 * h + 32, qlo:QT],
                         ones_sb[:], rhs,
                         start=first, stop=False,
                         tile_position=(0, 32 * h), skip_group_check=True)


def _bf(x):
    return np.ascontiguousarray(np.asarray(x, np.float32)).astype(ml_dtypes.bfloat16)


def _host_prep(inputs, T):
    content = np.asarray(inputs["content"], np.float32)
    category = np.asarray(inputs["category"], np.float32)
    Wqc = np.asarray(inputs["Wqc"], np.float32)
    Wkc = np.asarray(inputs["Wkc"], np.float32)
    Wv = np.asarray(inputs["Wv"], np.float32)
    Wqk = np.asarray(inputs["Wqk"], np.float32)
    Wkk = np.asarray(inputs["Wkk"], np.float32)
    Wo = np.asarray(inputs["Wo"], np.float32)
    bo = np.asarray(inputs["bo"], np.float32)
    alpha = 1.0 / (1.0 + np.exp(-float(np.asarray(inputs["alpha_logit"]))))
    nb = content.shape[0]

    scale_q = (HD ** -0.5) * LOG2E / KEXP

    def wchunks(W, s, scale=1.0):
        wt = (W.T * scale)[:, 128 * s:128 * (s + 1)]
        return _bf(wt.reshape(2, 128, 128))

    pio = np.arange(128, dtype=np.float32)[:, None]
    p_idx = np.arange(128)[:, None]
    qcol = np.arange(KT)[None, :]
    mzro = np.tile((qcol >= p_idx).astype(np.float32), (1, 2))
    mzro = _bf(mzro)

    in_maps = []
    for core in range(2 * nb):
        b, s = core // 2, core % 2
        m = {
            "xc": _bf(content[b].T.reshape(2, 128, T)),
            "xk": _bf(category[b].T.reshape(2, 128, T)),
            "wqc": wchunks(Wqc, s, scale_q),
            "wkc": wchunks(Wkc, s),
            "wqk": wchunks(Wqk, s, scale_q),
            "wkk": wchunks(Wkk, s),
            "wv": wchunks(Wv, s),
            "woc": _bf(Wo.T[128 * s:128 * (s + 1), :] * (1.0 - alpha)),
            "wok": _bf(Wo.T[128 * s:128 * (s + 1), :] * alpha),
            "bo_b": (np.tile(bo[None, :], (128, 1)) if s == 0
                     else np.zeros((128, 256), np.float32)),
            "pio": pio,
            "mzro": mzro,
        }
        in_maps.append(m)
    return in_maps


def _check_mask(mask, T):
    exp = np.triu(np.ones((T, T), dtype=bool), k=1)
    return np.array_equal(np.asarray(mask), exp)


def run(inputs, T=2048, cores=None, causal=True, **run_kwargs):
    """Build/compile (cached), run on hardware, return BassKernelResults."""
    key = (T, causal)
    if key not in _prog_cache:
        _prog_cache[key] = _build_program(T, causal=causal)
    nc = _prog_cache[key]
    in_maps = _host_prep(inputs, T)
    if cores is None:
        cores = list(range(len(in_maps)))
    res = run_bass_kernel_spmd(nc, [in_maps[c] for c in cores],
                               core_ids=list(range(len(cores))), **run_kwargs)
    return res


def kernel(**inputs):
    T = 2048
    mask = np.asarray(inputs["causal_mask"])
    if _check_mask(mask, T):
        causal = True
    elif not mask.any():
        causal = False
    else:
        raise NotImplementedError("kernel supports causal or empty masks only")
    res = run(inputs, T=T, causal=causal)
    nb = np.asarray(inputs["content"]).shape[0]
    out = np.empty((nb, T, D), np.float32)
    for b in range(nb):
        out[b] = res.results[2 * b]["y"] + res.results[2 * b + 1]["y"]
    return out



# revision 10
# speedup vs baseline: 1.2016x; 1.2016x over previous
"""DIF (dual-softmax) attention layer on 8 Trainium2 NeuronCores.

Sharding: core = (batch b, head-stack s), b in 0..3, s in 0..1.
Each core computes, for its batch and its 4 heads, the full dual-softmax
attention over all T rows, producing a partial output projection (sum over
its 4 heads). Host sums the two stack partials and adds the bias.

On-chip layout, q-tile=512, k-tile=128:
  - Q^T, K^T per branch: [128 (4h x 32d), T] bf16 in SBUF; attention scale
    * log2(e)/KEXP folded into the Q projection weights (exp runs in base 2).
  - scores (S^T layout): per (k-tile, head-pair) event, 2 row-packed matmuls
    (tile_position=(32h,0)) into one [128 k, 1024 (2h x 512q)] PSUM tile.
  - exp: split between ACT (activation Exp, scale=KEXP*ln2) and DVE
    (EXP2P poly + EXP2SQ5 squarings custom ops), greedily balanced at build
    time.  Causal triangles of diagonal k-tiles are zeroed post-exp by the
    Pool engine (mzro multiply, SBUF-only).
  - P@V: per (kt, head, q-chunk of 128): out O[128 q, 32 d] full-partition
    matmul (lhsT = exp-scores chunk, rhs = V tile), accumulating over kt in
    PSUM; denominator = same lhsT vs a ones column -> dn[128 q, 1].
    This makes PV+denom ~8x cheaper than col-packed O^T accumulation.
  - normalize: reciprocal(dn) broadcast-multiplied into O (stride-0 AP),
    PSUM -> SBUF bf16.
  - O^T for the output projection via PE transpose ([128,128] bf16 blocks)
    + DVE copy back to SBUF.
  - output projection: lhsT = O^T chunks, rhs = Wo^T slices pre-scaled by
    alpha / (1-alpha); both branches accumulate into one PSUM bank; copied
    out and DMA'd; bias is added on the host.
"""

import numpy as np
import ml_dtypes

import concourse.bass as bass
import concourse.tile as tile
from concourse import bacc, mybir, dve_ops
from concourse.dve_spec import (Spec, Src0, C0, C1, C2, C3, One, Idx,
                                lower, _spill_c3_to_src1, _has_src1 as has_src1)
from concourse.dve_uop import DveOpSpec
from concourse.bass_utils import run_bass_kernel_spmd

B, D, H, HD = 4, 256, 8, 32
HPS = 4  # heads per stack (per core)
LOG2E = 1.4426950408889634
LN2 = 0.6931471805599453
QT = 512  # q-tile width
KT = 128  # k-tile width
KEXP = 32  # exp2 split factor: exp2(y) = p(y/KEXP)^KEXP on the DVE path

# minimax coefficients for p(z) = 1 + z(a + z(b + z(c + d z))) ~ 2^z, |z|<=0.5
PA, PB, PC, PD = 0.693128038, 0.24023678, 0.055870371, 0.009590248

BF16 = mybir.dt.bfloat16
F32 = mybir.dt.float32
AF = mybir.ActivationFunctionType
OP = mybir.AluOpType

_prog_cache: dict = {}


def _register_dve_op(name, spec, subdim=False):
    """Register a custom DVE op at import time, self-pinning its uops sha."""
    for op in dve_ops.OPS:
        if op.name == name:
            return op
    row = dve_ops._CUSTOM_DVE_ROW_BASE + len(dve_ops.OPS)
    shas = {}
    for ver in ("v3", "v4"):
        s = DveOpSpec(name=name, opcode=row, uops=lower(spec, ver=ver),
                      rd1_en=has_src1(spec))
        shas[ver] = s.sha(ver)
    op = dve_ops.DveOp(name, spec, subdim=subdim, uops_sha=shas)
    dve_ops.OPS.append(op)
    dve_ops._SUB_OPCODE_FOR_NAME[name] = row
    dve_ops.CUSTOM_DVE_SPECS[name] = spec
    return op


def _make_exp_ops():
    z = Src0
    poly = One + z * (C0 + z * (C1 + z * (C2 + C3 * z)))
    k1 = _register_dve_op("ANT_EXP2_POLY", Spec(
        body=_spill_c3_to_src1(poly),
        reference=lambda in0, in1, s0, s1, imm2:
            1.0 + in0 * (s0 + in0 * (s1 + in0 * (
                imm2 + np.reshape(in1, (-1,) + (1,) * (in0.ndim - 1)) * in0))),
    ))
    p = Src0
    for _ in range(5):
        p = p * p
    k2 = _register_dve_op("ANT_EXP2_SQ5", Spec(
        body=p,
        reference=lambda in0, in1, s0, s1, imm2: in0.astype(np.float32) ** 32,
    ))
    return k1, k2


EXP2P, EXP2SQ5 = _make_exp_ops()

# engine-balance cost constants (ns), from the TRN2 instruction cost model
_ACT_COL = 0.8333
_ACT_OVH = 190.0
_DVE_COL = 2.0833   # two custom-op passes
_DVE_OVH = 250.0


def _build_program(T, causal=True):
    nc = bacc.Bacc("TRN2", target_bir_lowering=False, debug=False)

    xc = nc.dram_tensor("xc", [2, 128, T], BF16, kind="ExternalInput")
    xk = nc.dram_tensor("xk", [2, 128, T], BF16, kind="ExternalInput")
    wqc = nc.dram_tensor("wqc", [2, 128, 128], BF16, kind="ExternalInput")
    wkc = nc.dram_tensor("wkc", [2, 128, 128], BF16, kind="ExternalInput")
    wqk = nc.dram_tensor("wqk", [2, 128, 128], BF16, kind="ExternalInput")
    wkk = nc.dram_tensor("wkk", [2, 128, 128], BF16, kind="ExternalInput")
    wv = nc.dram_tensor("wv", [2, 128, 128], BF16, kind="ExternalInput")
    woc = nc.dram_tensor("woc", [128, 256], BF16, kind="ExternalInput")
    wok = nc.dram_tensor("wok", [128, 256], BF16, kind="ExternalInput")
    ident = nc.dram_tensor("ident", [128, 128], BF16, kind="ExternalInput")
    mzro = nc.dram_tensor("mzro", [128, 2 * KT], BF16, kind="ExternalInput")
    y = nc.dram_tensor("y", [T, 256], F32, kind="ExternalOutput")

    NQT = T // QT
    NTT = T // KT
    NCH = QT // 128  # q-chunks per q-tile

    # build-time greedy engine balance for the exp events
    eng_t = {"act": 0.0, "dve": 0.0}

    import os
    force = os.environ.get("EXP_ENGINE", "")

    def pick_exp_engine(cols):
        if force:
            return force
        fa = eng_t["act"] + cols * _ACT_COL + _ACT_OVH
        fd = eng_t["dve"] + cols * _DVE_COL + _DVE_OVH
        if fa <= fd:
            eng_t["act"] = fa
            return "act"
        eng_t["dve"] = fd
        return "dve"

    with tile.TileContext(nc) as tc:
        with (
            tc.tile_pool(name="xin", bufs=1) as xin,
            tc.tile_pool(name="wts", bufs=1) as wts,
            tc.tile_pool(name="proj", bufs=1) as proj,
            tc.tile_pool(name="exps", bufs=4) as exps,
            tc.tile_pool(name="ex1p", bufs=3) as ex1p,
            tc.tile_pool(name="onrm", bufs=2) as onrm,
            tc.tile_pool(name="otsb", bufs=10) as otsb,
            tc.tile_pool(name="recp", bufs=2) as recp,
            tc.tile_pool(name="yout", bufs=3) as yout,
            tc.tile_pool(name="ps_sc", bufs=2, space="PSUM") as ps_sc,
            tc.tile_pool(name="ps_o", bufs=2, space="PSUM") as ps_o,
            tc.tile_pool(name="ps_dn", bufs=1, space="PSUM") as ps_dn,
            tc.tile_pool(name="ps_mm", bufs=1, space="PSUM") as ps_mm,
        ):
            # ---- load inputs ----
            xc0 = xin.tile([128, T], BF16, tag="xc0")
            xc1 = xin.tile([128, T], BF16, tag="xc1")
            xk0 = xin.tile([128, T], BF16, tag="xk0")
            xk1 = xin.tile([128, T], BF16, tag="xk1")
            nch = 2 if T >= 1024 else 1
            for ch in range(nch):
                sl = bass.ts(ch, T // nch)
                nc.sync.dma_start(xc0[:, sl], xc[0][:, sl])
                nc.sync.dma_start(xc1[:, sl], xc[1][:, sl])
                nc.sync.dma_start(xk0[:, sl], xk[0][:, sl])
                nc.sync.dma_start(xk1[:, sl], xk[1][:, sl])

            # warm the ACT exp table while DMAs stream in
            warm = wts.tile([128, 1], F32, tag="warm")
            nc.vector.memset(warm[:], 0.0)
            nc.scalar.activation(warm[:], warm[:], AF.Exp, scale=1.0)

            w_sb = {}
            for nm, dram in [("wqc", wqc), ("wkc", wkc), ("wqk", wqk),
                             ("wkk", wkk), ("wv", wv)]:
                for j in range(2):
                    t = wts.tile([128, 128], BF16, tag=f"{nm}{j}")
                    nc.sync.dma_start(t[:], dram[j])
                    w_sb[(nm, j)] = t
            woc_sb = wts.tile([128, 256], BF16, tag="woc")
            wok_sb = wts.tile([128, 256], BF16, tag="wok")
            id_sb = wts.tile([128, 128], BF16, tag="ident")
            mzro_sb = wts.tile([128, 2 * KT], BF16, tag="mzro")
            nc.sync.dma_start(woc_sb[:], woc[:])
            nc.sync.dma_start(wok_sb[:], wok[:])
            nc.sync.dma_start(id_sb[:], ident[:])
            nc.sync.dma_start(mzro_sb[:], mzro[:])
            ones_sb = wts.tile([128, 1], BF16, tag="ones")
            nc.vector.memset(ones_sb[:], 1.0)
            dco_sb = wts.tile([128, 1], F32, tag="dco")
            nc.vector.memset(dco_sb[:], PD)

            # ---- projections ----
            # Q^T/K^T: out[i, t] = sum_j W.T[j, i] * x^T[j, t]
            qkt = {}
            for pi, (nm, xs) in enumerate([
                    ("wqc", (xc0, xc1)), ("wkc", (xc0, xc1)),
                    ("wqk", (xk0, xk1)), ("wkk", (xk0, xk1))]):
                dst = proj.tile([128, T], BF16, tag=f"p_{nm}")
                qkt[nm] = dst
                for nt in range(T // 512):
                    ps = ps_mm.tile([128, 512], F32, tag="mm")
                    sl = bass.ts(nt, 512)
                    nc.tensor.matmul(ps[:], w_sb[(nm, 0)][:], xs[0][:, sl],
                                     start=True, stop=False)
                    nc.tensor.matmul(ps[:], w_sb[(nm, 1)][:], xs[1][:, sl],
                                     start=False, stop=True)
                    if (pi * (T // 512) + nt) % 2 == 0:
                        nc.vector.tensor_copy(dst[:, sl], ps[:])
                    else:
                        nc.scalar.copy(dst[:, sl], ps[:])
            q_c, k_c, q_k, k_k = qkt["wqc"], qkt["wkc"], qkt["wqk"], qkt["wkk"]

            # V: out[t, i] = sum_j x^T[j, t] * Wv.T[j, i]; layout [128, NTT, 128]
            v_sb = proj.tile([128, NTT, 128], BF16, tag="p_v")
            for tt in range(NTT):
                ps = ps_mm.tile([128, 512], F32, tag="mm")
                sl = bass.ts(tt, 128)
                nc.tensor.matmul(ps[:, 0:128], xc0[:, sl], w_sb[("wv", 0)][:],
                                 start=True, stop=False)
                nc.tensor.matmul(ps[:, 0:128], xc1[:, sl], w_sb[("wv", 1)][:],
                                 start=False, stop=True)
                if tt % 2 == 0:
                    nc.scalar.copy(v_sb[:, tt, :], ps[:, 0:128])
                else:
                    nc.vector.tensor_copy(v_sb[:, tt, :], ps[:, 0:128])

            # ---- attention ----
            # deferred PE ops (transposes / out-proj) flushed between events
            pending = []

            def flush_pending(n=1):
                for _ in range(min(n, len(pending))):
                    pending.pop(0)()

            for qt in range(NQT):
                q0 = qt * QT
                nkt = (q0 + QT) // KT if causal else NTT
                nmask = 4 if causal else 0
                kts_clean = list(range(nkt - nmask))
                kts_mask = list(range(nkt - nmask, nkt))
                # interleave diagonal k-tiles among the clean ones
                order = []
                if kts_clean:
                    stride = max(1, len(kts_clean) // 4)
                    mi = 0
                    for i, kt in enumerate(kts_clean):
                        order.append(kt)
                        if (i + 1) % stride == 0 and mi < nmask:
                            order.append(kts_mask[mi])
                            mi += 1
                    order += kts_mask[mi:]
                else:
                    order = kts_mask

                ot_tiles = {}  # (br, m) -> O^T sbuf chunk
                for br, (qsb, ksb) in [("c", (q_c, k_c)), ("k", (q_k, k_k))]:
                    o_ps = ps_o.tile([128, NCH, 128], F32, tag="o")
                    dn_ps = ps_dn.tile([128, NCH, HPS], F32, tag="dn")
                    started = set()
                    # last event index per chunk (for stop flags)
                    last_ev = {}
                    for ei, kt in enumerate(order):
                        di = kt - (nkt - 4) if causal else -1
                        c0 = di if di > 0 else 0
                        for c in range(c0, NCH):
                            last_ev[c] = ei
                    pend = None
                    for ei, kt in enumerate(order):
                        k0 = kt * KT
                        di = kt - (nkt - 4) if causal else -1
                        qlo = 128 * di if di > 0 else 0
                        for hp in range(2):
                            sp = ps_sc.tile([128, 2 * QT], F32, tag="sc")
                            for hl in range(2):
                                h = 2 * hp + hl
                                nc.tensor.matmul(
                                    sp[:, QT * hl + qlo: QT * (hl + 1)],
                                    ksb[32 * h:32 * h + 32, k0:k0 + KT],
                                    qsb[32 * h:32 * h + 32, q0 + qlo:q0 + QT],
                                    start=True, stop=True,
                                    tile_position=(32 * h, 0),
                                    skip_group_check=True,
                                )
                            ex = exps.tile([128, 2 * QT], BF16, tag="ex")
                            cols = 2 * (QT - qlo)
                            eng = pick_exp_engine(cols)
                            if eng == "dve":
                                e1 = ex1p.tile([128, 2 * QT], F32, tag="e1")
                                if qlo:
                                    spv = sp[:].rearrange(
                                        "p (l q) -> p l q", l=2)[:, :, qlo:]
                                    e1v = e1[:].rearrange(
                                        "p (l q) -> p l q", l=2)[:, :, qlo:]
                                    exv = ex[:].rearrange(
                                        "p (l q) -> p l q", l=2)[:, :, qlo:]
                                else:
                                    spv, e1v, exv = sp[:], e1[:], ex[:]
                                nc.vector._custom_dve(
                                    EXP2P, out=e1v, in0=spv,
                                    in1=dco_sb[:, 0:1], s0=PA, s1=PB, imm2=PC)
                                nc.vector._custom_dve(EXP2SQ5, out=exv, in0=e1v)
                            else:
                                if qlo:
                                    nc.scalar.activation(
                                        ex[:].rearrange("p (l q) -> p l q", l=2)[:, :, qlo:],
                                        sp[:].rearrange("p (l q) -> p l q", l=2)[:, :, qlo:],
                                        AF.Exp, scale=KEXP * LN2)
                                else:
                                    nc.scalar.activation(ex[:], sp[:], AF.Exp,
                                                         scale=KEXP * LN2)
                            if di >= 0:
                                # zero the causal triangle post-exp (Pool)
                                exv = ex[:].rearrange(
                                    "p (l q) -> p l q", l=2)[:, :, qlo:qlo + KT]
                                nc.gpsimd.tensor_tensor(
                                    exv, exv,
                                    mzro_sb[:].rearrange("p (l q) -> p l q", l=2),
                                    OP.mult)
                            if pend is not None:
                                _pv_dn(nc, o_ps, dn_ps, v_sb, ones_sb,
                                       started, last_ev, *pend)
                            flush_pending(1)
                            pend = (ex, kt, hp, di, ei)
                    _pv_dn(nc, o_ps, dn_ps, v_sb, ones_sb, started, last_ev,
                           *pend)

                    # normalize + transpose (deferred PE work via pending)
                    def norm_and_tp(br=br, o_ps=o_ps, dn_ps=dn_ps,
                                    ot_tiles=ot_tiles):
                        rec = recp.tile([128, NCH, HPS], F32, tag="rec")
                        nc.vector.reciprocal_approx_fast(
                            rec[:].rearrange("p c h -> p (c h)"),
                            dn_ps[:].rearrange("p c h -> p (c h)"))
                        eng_t["dve"] += 140
                        on = onrm.tile([128, NCH, 128], BF16, tag=f"on{br}")
                        rec_bc = rec[:].unsqueeze(3).broadcast_to(
                            [128, NCH, HPS, 32])
                        nc.vector.tensor_tensor(
                            on[:].rearrange("p c (h d) -> p c h d", h=HPS),
                            o_ps[:].rearrange("p c (h d) -> p c h d", h=HPS),
                            rec_bc, OP.mult)
                        eng_t["dve"] += 660
                        for m in range(NCH):
                            def tp(m=m, on=on, br=br, ot_tiles=ot_tiles):
                                ot_ps = ps_mm.tile([128, 1024], BF16, tag="mm")
                                nc.tensor.matmul(ot_ps[:, 0:128], on[:, m, :],
                                                 id_sb[:], is_transpose=True,
                                                 start=True, stop=True)
                                ot = otsb.tile([128, 128], BF16, tag="ot")
                                nc.vector.tensor_copy(ot[:], ot_ps[:, 0:128])
                                eng_t["dve"] += 200
                                ot_tiles[(br, m)] = ot
                            pending.append(tp)
                    norm_and_tp()

                # out-projection, deferred between next events
                for m in range(NCH):
                    def outp(m=m, qt=qt, q0=q0, ot_tiles=ot_tiles):
                        yp = ps_mm.tile([128, 512], F32, tag="mm")
                        nc.tensor.matmul(yp[:, 0:256], ot_tiles[("c", m)][:],
                                         woc_sb[:], start=True, stop=False)
                        nc.tensor.matmul(yp[:, 0:256], ot_tiles[("k", m)][:],
                                         wok_sb[:], start=False, stop=True)
                        ysb = yout.tile([128, 256], F32, tag="y")
                        nc.scalar.copy(ysb[:], yp[:, 0:256])
                        eng_t["act"] += 400
                        nc.sync.dma_start(
                            y[q0 + m * 128:q0 + (m + 1) * 128, :], ysb[:])
                    pending.append(outp)
            flush_pending(len(pending))

    nc.compile()
    return nc


def _pv_dn(nc, o_ps, dn_ps, v_sb, ones_sb, started, last_ev, ex, kt, hp, di,
           ei):
    # PSUM "start" marks the whole 2KB zero region pending-zero, so exactly
    # one matmul per PSUM tile may carry start=True; every other group's
    # first write is auto-zeroed by that region-wide pending state.
    c0 = di if di > 0 else 0
    ei_end = max(last_ev.values())
    for hl in range(2):
        h = 2 * hp + hl
        for c in range(c0, 4):
            stop = ei == ei_end
            lhsT = ex[:, QT * hl + c * 128: QT * hl + (c + 1) * 128]
            nc.tensor.matmul(o_ps[:, c, 32 * h:32 * h + 32],
                             lhsT, v_sb[:, kt, 32 * h:32 * h + 32],
                             start="o" not in started, stop=stop,
                             skip_group_check=True)
            started.add("o")
            nc.tensor.matmul(dn_ps[:, c, h:h + 1],
                             lhsT, ones_sb[:],
                             start="dn" not in started, stop=stop,
                             skip_group_check=True)
            started.add("dn")


def _bf(x):
    return np.ascontiguousarray(np.asarray(x, np.float32)).astype(ml_dtypes.bfloat16)


def _host_prep(inputs, T):
    content = np.asarray(inputs["content"], np.float32)
    category = np.asarray(inputs["category"], np.float32)
    Wqc = np.asarray(inputs["Wqc"], np.float32)
    Wkc = np.asarray(inputs["Wkc"], np.float32)
    Wv = np.asarray(inputs["Wv"], np.float32)
    Wqk = np.asarray(inputs["Wqk"], np.float32)
    Wkk = np.asarray(inputs["Wkk"], np.float32)
    Wo = np.asarray(inputs["Wo"], np.float32)
    alpha = 1.0 / (1.0 + np.exp(-float(np.asarray(inputs["alpha_logit"]))))
    nb = content.shape[0]

    scale_q = (HD ** -0.5) * LOG2E / KEXP

    def wchunks(W, s, scale=1.0):
        wt = (W.T * scale)[:, 128 * s:128 * (s + 1)]
        return _bf(wt.reshape(2, 128, 128))

    p_idx = np.arange(128)[:, None]
    qcol = np.arange(KT)[None, :]
    mzro = np.tile((qcol >= p_idx).astype(np.float32), (1, 2))
    mzro = _bf(mzro)
    ident = _bf(np.eye(128, dtype=np.float32))

    in_maps = []
    for core in range(2 * nb):
        b, s = core // 2, core % 2
        m = {
            "xc": _bf(content[b].T.reshape(2, 128, T)),
            "xk": _bf(category[b].T.reshape(2, 128, T)),
            "wqc": wchunks(Wqc, s, scale_q),
            "wkc": wchunks(Wkc, s),
            "wqk": wchunks(Wqk, s, scale_q),
            "wkk": wchunks(Wkk, s),
            "wv": wchunks(Wv, s),
            "woc": _bf(Wo.T[128 * s:128 * (s + 1), :] * (1.0 - alpha)),
            "wok": _bf(Wo.T[128 * s:128 * (s + 1), :] * alpha),
            "ident": ident,
            "mzro": mzro,
        }
        in_maps.append(m)
    return in_maps


def _check_mask(mask, T):
    exp = np.triu(np.ones((T, T), dtype=bool), k=1)
    return np.array_equal(np.asarray(mask), exp)


def run(inputs, T=2048, cores=None, causal=True, **run_kwargs):
    """Build/compile (cached), run on hardware, return BassKernelResults."""
    key = (T, causal)
    if key not in _prog_cache:
        _prog_cache[key] = _build_program(T, causal=causal)
    nc = _prog_cache[key]
    in_maps = _host_prep(inputs, T)
    if cores is None:
        cores = list(range(len(in_maps)))
    res = run_bass_kernel_spmd(nc, [in_maps[c] for c in cores],
                               core_ids=list(range(len(cores))), **run_kwargs)
    return res


def kernel(**inputs):
    T = 2048
    mask = np.asarray(inputs["causal_mask"])
    if _check_mask(mask, T):
        causal = True
    elif not mask.any():
        causal = False
    else:
        raise NotImplementedError("kernel supports causal or empty masks only")
    res = run(inputs, T=T, causal=causal)
    nb = np.asarray(inputs["content"]).shape[0]
    bo = np.asarray(inputs["bo"], np.float32)
    out = np.empty((nb, T, D), np.float32)
    for b in range(nb):
        out[b] = res.results[2 * b]["y"] + res.results[2 * b + 1]["y"] + bo
    return out


# revision 11
# speedup vs baseline: 1.2213x; 1.0163x over previous
"""DIF (dual-softmax) attention layer on 8 Trainium2 NeuronCores.

Sharding: core = (batch b, head-stack s), b in 0..3, s in 0..1.
Each core computes, for its batch and its 4 heads, the full dual-softmax
attention over all T rows, producing a partial output projection (sum over
its 4 heads). Host sums the two stack partials and adds the bias.

On-chip layout, q-tile=512, k-tile=128:
  - Q^T, K^T per branch: [128 (4h x 32d), T] bf16 in SBUF; attention scale
    * log2(e)/KEXP folded into the Q projection weights (exp runs in base 2).
  - scores (S^T layout): per (k-tile, head-pair) event, 2 row-packed matmuls
    (tile_position=(32h,0)) into one [128 k, 1024 (2h x 512q)] PSUM tile.
  - exp: split between ACT (activation Exp, scale=KEXP*ln2) and DVE
    (EXP2P poly + EXP2SQ5 squarings custom ops), greedily balanced at build
    time.  Causal triangles of diagonal k-tiles are zeroed post-exp by the
    Pool engine (mzro multiply, SBUF-only).
  - P@V: per (kt, head, q-chunk of 128): out O[128 q, 32 d] full-partition
    matmul (lhsT = exp-scores chunk, rhs = V tile), accumulating over kt in
    PSUM; denominator = same lhsT vs a ones column -> dn[128 q, 1].
    This makes PV+denom ~8x cheaper than col-packed O^T accumulation.
  - normalize: reciprocal(dn) broadcast-multiplied into O (stride-0 AP),
    PSUM -> SBUF bf16.
  - O^T for the output projection via PE transpose ([128,128] bf16 blocks)
    + DVE copy back to SBUF.
  - output projection: lhsT = O^T chunks, rhs = Wo^T slices pre-scaled by
    alpha / (1-alpha); both branches accumulate into one PSUM bank; copied
    out and DMA'd; bias is added on the host.
"""

import numpy as np
import ml_dtypes

import concourse.bass as bass
import concourse.tile as tile
from concourse import bacc, mybir, dve_ops
from concourse.dve_spec import (Spec, Src0, C0, C1, C2, C3, One, Idx,
                                lower, _spill_c3_to_src1, _has_src1 as has_src1)
from concourse.dve_uop import DveOpSpec
from concourse.bass_utils import run_bass_kernel_spmd

B, D, H, HD = 4, 256, 8, 32
HPS = 4  # heads per stack (per core)
LOG2E = 1.4426950408889634
LN2 = 0.6931471805599453
QT = 512  # q-tile width
KT = 128  # k-tile width
KEXP = 32  # exp2 split factor: exp2(y) = p(y/KEXP)^KEXP on the DVE path

# minimax coefficients for p(z) = 1 + z(a + z(b + z(c + d z))) ~ 2^z, |z|<=0.5
PA, PB, PC, PD = 0.693128038, 0.24023678, 0.055870371, 0.009590248

BF16 = mybir.dt.bfloat16
F32 = mybir.dt.float32
AF = mybir.ActivationFunctionType
OP = mybir.AluOpType

_prog_cache: dict = {}


def _register_dve_op(name, spec, subdim=False):
    """Register a custom DVE op at import time, self-pinning its uops sha."""
    for op in dve_ops.OPS:
        if op.name == name:
            return op
    row = dve_ops._CUSTOM_DVE_ROW_BASE + len(dve_ops.OPS)
    shas = {}
    for ver in ("v3", "v4"):
        s = DveOpSpec(name=name, opcode=row, uops=lower(spec, ver=ver),
                      rd1_en=has_src1(spec))
        shas[ver] = s.sha(ver)
    op = dve_ops.DveOp(name, spec, subdim=subdim, uops_sha=shas)
    dve_ops.OPS.append(op)
    dve_ops._SUB_OPCODE_FOR_NAME[name] = row
    dve_ops.CUSTOM_DVE_SPECS[name] = spec
    return op


def _make_exp_ops():
    z = Src0
    poly = One + z * (C0 + z * (C1 + z * (C2 + C3 * z)))
    k1 = _register_dve_op("ANT_EXP2_POLY", Spec(
        body=_spill_c3_to_src1(poly),
        reference=lambda in0, in1, s0, s1, imm2:
            1.0 + in0 * (s0 + in0 * (s1 + in0 * (
                imm2 + np.reshape(in1, (-1,) + (1,) * (in0.ndim - 1)) * in0))),
    ))
    p = Src0
    for _ in range(5):
        p = p * p
    k2 = _register_dve_op("ANT_EXP2_SQ5", Spec(
        body=p,
        reference=lambda in0, in1, s0, s1, imm2: in0.astype(np.float32) ** 32,
    ))
    return k1, k2


EXP2P, EXP2SQ5 = _make_exp_ops()

# engine-balance cost constants (ns), from the TRN2 instruction cost model
_ACT_COL = 0.8333
_ACT_OVH = 190.0
_DVE_COL = 2.0833   # two custom-op passes
_DVE_OVH = 250.0


def _build_program(T, causal=True):
    nc = bacc.Bacc("TRN2", target_bir_lowering=False, debug=False)

    xc = nc.dram_tensor("xc", [2, 128, T], BF16, kind="ExternalInput")
    xk = nc.dram_tensor("xk", [2, 128, T], BF16, kind="ExternalInput")
    wqc = nc.dram_tensor("wqc", [2, 128, 128], BF16, kind="ExternalInput")
    wkc = nc.dram_tensor("wkc", [2, 128, 128], BF16, kind="ExternalInput")
    wqk = nc.dram_tensor("wqk", [2, 128, 128], BF16, kind="ExternalInput")
    wkk = nc.dram_tensor("wkk", [2, 128, 128], BF16, kind="ExternalInput")
    wv = nc.dram_tensor("wv", [2, 128, 128], BF16, kind="ExternalInput")
    woc = nc.dram_tensor("woc", [128, 256], BF16, kind="ExternalInput")
    wok = nc.dram_tensor("wok", [128, 256], BF16, kind="ExternalInput")
    ident = nc.dram_tensor("ident", [128, 128], BF16, kind="ExternalInput")
    mzro = nc.dram_tensor("mzro", [128, 2 * KT], BF16, kind="ExternalInput")
    y = nc.dram_tensor("y", [T, 256], F32, kind="ExternalOutput")

    NQT = T // QT
    NTT = T // KT
    NCH = QT // 128  # q-chunks per q-tile

    # build-time greedy engine balance for the exp events
    eng_t = {"act": 0.0, "dve": 0.0}

    import os
    force = os.environ.get("EXP_ENGINE", "")

    def pick_exp_engine(cols):
        if force:
            return force
        fa = eng_t["act"] + cols * _ACT_COL + _ACT_OVH
        fd = eng_t["dve"] + cols * _DVE_COL + _DVE_OVH
        if fa <= fd:
            eng_t["act"] = fa
            return "act"
        eng_t["dve"] = fd
        return "dve"

    with tile.TileContext(nc) as tc:
        with (
            tc.tile_pool(name="xin", bufs=1) as xin,
            tc.tile_pool(name="wts", bufs=1) as wts,
            tc.tile_pool(name="proj", bufs=1) as proj,
            tc.tile_pool(name="exps", bufs=4) as exps,
            tc.tile_pool(name="ex1p", bufs=3) as ex1p,
            tc.tile_pool(name="onrm", bufs=2) as onrm,
            tc.tile_pool(name="otsb", bufs=10) as otsb,
            tc.tile_pool(name="recp", bufs=2) as recp,
            tc.tile_pool(name="yout", bufs=3) as yout,
            tc.tile_pool(name="ps_sc", bufs=2, space="PSUM") as ps_sc,
            tc.tile_pool(name="ps_o", bufs=2, space="PSUM") as ps_o,
            tc.tile_pool(name="ps_dn", bufs=1, space="PSUM") as ps_dn,
            tc.tile_pool(name="ps_mm", bufs=1, space="PSUM") as ps_mm,
        ):
            # ---- load inputs ----
            xc0 = xin.tile([128, T], BF16, tag="xc0")
            xc1 = xin.tile([128, T], BF16, tag="xc1")
            xk0 = xin.tile([128, T], BF16, tag="xk0")
            xk1 = xin.tile([128, T], BF16, tag="xk1")
            nch = 2 if T >= 1024 else 1
            for ch in range(nch):
                sl = bass.ts(ch, T // nch)
                nc.sync.dma_start(xc0[:, sl], xc[0][:, sl])
                nc.sync.dma_start(xc1[:, sl], xc[1][:, sl])
                nc.sync.dma_start(xk0[:, sl], xk[0][:, sl])
                nc.sync.dma_start(xk1[:, sl], xk[1][:, sl])

            # warm the ACT exp table while DMAs stream in
            warm = wts.tile([128, 1], F32, tag="warm")
            nc.vector.memset(warm[:], 0.0)
            nc.scalar.activation(warm[:], warm[:], AF.Exp, scale=1.0)

            w_sb = {}
            for nm, dram in [("wqc", wqc), ("wkc", wkc), ("wqk", wqk),
                             ("wkk", wkk), ("wv", wv)]:
                for j in range(2):
                    t = wts.tile([128, 128], BF16, tag=f"{nm}{j}")
                    nc.sync.dma_start(t[:], dram[j])
                    w_sb[(nm, j)] = t
            woc_sb = wts.tile([128, 256], BF16, tag="woc")
            wok_sb = wts.tile([128, 256], BF16, tag="wok")
            id_sb = wts.tile([128, 128], BF16, tag="ident")
            mzro_sb = wts.tile([128, 2 * KT], BF16, tag="mzro")
            nc.sync.dma_start(woc_sb[:], woc[:])
            nc.sync.dma_start(wok_sb[:], wok[:])
            nc.sync.dma_start(id_sb[:], ident[:])
            nc.sync.dma_start(mzro_sb[:], mzro[:])
            ones_sb = wts.tile([128, 1], BF16, tag="ones")
            nc.vector.memset(ones_sb[:], 1.0)
            dco_sb = wts.tile([128, 1], F32, tag="dco")
            nc.vector.memset(dco_sb[:], PD)

            # ---- projections ----
            # Q^T/K^T: out[i, t] = sum_j W.T[j, i] * x^T[j, t]
            qkt = {}
            for pi, (nm, xs) in enumerate([
                    ("wqc", (xc0, xc1)), ("wkc", (xc0, xc1)),
                    ("wqk", (xk0, xk1)), ("wkk", (xk0, xk1))]):
                dst = proj.tile([128, T], BF16, tag=f"p_{nm}")
                qkt[nm] = dst
                for nt in range(T // 512):
                    ps = ps_mm.tile([128, 512], F32, tag="mm")
                    sl = bass.ts(nt, 512)
                    nc.tensor.matmul(ps[:], w_sb[(nm, 0)][:], xs[0][:, sl],
                                     start=True, stop=False)
                    nc.tensor.matmul(ps[:], w_sb[(nm, 1)][:], xs[1][:, sl],
                                     start=False, stop=True)
                    if (pi * (T // 512) + nt) % 2 == 0:
                        nc.vector.tensor_copy(dst[:, sl], ps[:])
                    else:
                        nc.scalar.copy(dst[:, sl], ps[:])
            q_c, k_c, q_k, k_k = qkt["wqc"], qkt["wkc"], qkt["wqk"], qkt["wkk"]

            # V: out[t, i] = sum_j x^T[j, t] * Wv.T[j, i]; layout [128, NTT, 128]
            v_sb = proj.tile([128, NTT, 128], BF16, tag="p_v")
            for tt in range(NTT):
                ps = ps_mm.tile([128, 512], F32, tag="mm")
                sl = bass.ts(tt, 128)
                nc.tensor.matmul(ps[:, 0:128], xc0[:, sl], w_sb[("wv", 0)][:],
                                 start=True, stop=False)
                nc.tensor.matmul(ps[:, 0:128], xc1[:, sl], w_sb[("wv", 1)][:],
                                 start=False, stop=True)
                if tt % 2 == 0:
                    nc.scalar.copy(v_sb[:, tt, :], ps[:, 0:128])
                else:
                    nc.vector.tensor_copy(v_sb[:, tt, :], ps[:, 0:128])

            # ---- attention ----
            # deferred PE ops (transposes / out-proj) flushed between events
            pending = []

            def flush_pending(n=1):
                for _ in range(min(n, len(pending))):
                    pending.pop(0)()

            for qt in range(NQT):
                q0 = qt * QT
                nkt = (q0 + QT) // KT if causal else NTT
                nmask = 4 if causal else 0
                kts_clean = list(range(nkt - nmask))
                kts_mask = list(range(nkt - nmask, nkt))
                # interleave diagonal k-tiles among the clean ones
                order = []
                if kts_clean:
                    stride = max(1, len(kts_clean) // 4)
                    mi = 0
                    for i, kt in enumerate(kts_clean):
                        order.append(kt)
                        if (i + 1) % stride == 0 and mi < nmask:
                            order.append(kts_mask[mi])
                            mi += 1
                    order += kts_mask[mi:]
                else:
                    order = kts_mask

                ot_tiles = {}  # (br, m) -> O^T sbuf chunk
                for br, (qsb, ksb) in [("c", (q_c, k_c)), ("k", (q_k, k_k))]:
                    o_ps = ps_o.tile([128, NCH, 128], F32, tag="o")
                    dn_ps = ps_dn.tile([128, NCH, HPS], F32, tag="dn")
                    started = set()
                    # last event index per chunk (for stop flags)
                    last_ev = {}
                    for ei, kt in enumerate(order):
                        di = kt - (nkt - 4) if causal else -1
                        c0 = di if di > 0 else 0
                        for c in range(c0, NCH):
                            last_ev[c] = ei
                    pend = []
                    for ei, kt in enumerate(order):
                        k0 = kt * KT
                        di = kt - (nkt - 4) if causal else -1
                        qlo = 128 * di if di > 0 else 0
                        for hp in range(2):
                            sp = ps_sc.tile([128, 2 * QT], F32, tag="sc")
                            for hl in range(2):
                                h = 2 * hp + hl
                                nc.tensor.matmul(
                                    sp[:, QT * hl + qlo: QT * (hl + 1)],
                                    ksb[32 * h:32 * h + 32, k0:k0 + KT],
                                    qsb[32 * h:32 * h + 32, q0 + qlo:q0 + QT],
                                    start=True, stop=True,
                                    tile_position=(32 * h, 0),
                                    skip_group_check=True,
                                )
                            ex = exps.tile([128, 2 * QT], BF16, tag="ex")
                            cols = 2 * (QT - qlo)
                            eng = pick_exp_engine(cols)
                            if eng == "dve":
                                e1 = ex1p.tile([128, 2 * QT], F32, tag="e1")
                                if qlo:
                                    spv = sp[:].rearrange(
                                        "p (l q) -> p l q", l=2)[:, :, qlo:]
                                    e1v = e1[:].rearrange(
                                        "p (l q) -> p l q", l=2)[:, :, qlo:]
                                    exv = ex[:].rearrange(
                                        "p (l q) -> p l q", l=2)[:, :, qlo:]
                                else:
                                    spv, e1v, exv = sp[:], e1[:], ex[:]
                                nc.vector._custom_dve(
                                    EXP2P, out=e1v, in0=spv,
                                    in1=dco_sb[:, 0:1], s0=PA, s1=PB, imm2=PC)
                                nc.vector._custom_dve(EXP2SQ5, out=exv, in0=e1v)
                            else:
                                if qlo:
                                    nc.scalar.activation(
                                        ex[:].rearrange("p (l q) -> p l q", l=2)[:, :, qlo:],
                                        sp[:].rearrange("p (l q) -> p l q", l=2)[:, :, qlo:],
                                        AF.Exp, scale=KEXP * LN2)
                                else:
                                    nc.scalar.activation(ex[:], sp[:], AF.Exp,
                                                         scale=KEXP * LN2)
                            if di >= 0:
                                # zero the causal triangle post-exp (Pool)
                                exv = ex[:].rearrange(
                                    "p (l q) -> p l q", l=2)[:, :, qlo:qlo + KT]
                                nc.gpsimd.tensor_tensor(
                                    exv, exv,
                                    mzro_sb[:].rearrange("p (l q) -> p l q", l=2),
                                    OP.mult)
                            if len(pend) == 2:
                                _pv_dn(nc, o_ps, dn_ps, v_sb, ones_sb,
                                       started, last_ev, *pend.pop(0))
                            flush_pending(1)
                            pend.append((ex, kt, hp, di, ei))
                    for pe in pend:
                        _pv_dn(nc, o_ps, dn_ps, v_sb, ones_sb, started,
                               last_ev, *pe)

                    # normalize + transpose (deferred PE work via pending)
                    def norm_and_tp(br=br, o_ps=o_ps, dn_ps=dn_ps,
                                    ot_tiles=ot_tiles):
                        rec = recp.tile([128, NCH, HPS], F32, tag="rec")
                        nc.vector.reciprocal_approx_fast(
                            rec[:].rearrange("p c h -> p (c h)"),
                            dn_ps[:].rearrange("p c h -> p (c h)"))
                        eng_t["dve"] += 140
                        on = onrm.tile([128, NCH, 128], BF16, tag=f"on{br}")
                        rec_bc = rec[:].unsqueeze(3).broadcast_to(
                            [128, NCH, HPS, 32])
                        nc.vector.tensor_tensor(
                            on[:].rearrange("p c (h d) -> p c h d", h=HPS),
                            o_ps[:].rearrange("p c (h d) -> p c h d", h=HPS),
                            rec_bc, OP.mult)
                        eng_t["dve"] += 660
                        for m in range(NCH):
                            def tp(m=m, on=on, br=br, ot_tiles=ot_tiles):
                                ot_ps = ps_mm.tile([128, 1024], BF16, tag="mm")
                                nc.tensor.matmul(ot_ps[:, 0:128], on[:, m, :],
                                                 id_sb[:], is_transpose=True,
                                                 start=True, stop=True)
                                ot = otsb.tile([128, 128], BF16, tag="ot")
                                nc.vector.tensor_copy(ot[:], ot_ps[:, 0:128])
                                eng_t["dve"] += 200
                                ot_tiles[(br, m)] = ot
                            pending.append(tp)
                    norm_and_tp()

                # out-projection, deferred between next events
                for m in range(NCH):
                    def outp(m=m, qt=qt, q0=q0, ot_tiles=ot_tiles):
                        yp = ps_mm.tile([128, 512], F32, tag="mm")
                        nc.tensor.matmul(yp[:, 0:256], ot_tiles[("c", m)][:],
                                         woc_sb[:], start=True, stop=False)
                        nc.tensor.matmul(yp[:, 0:256], ot_tiles[("k", m)][:],
                                         wok_sb[:], start=False, stop=True)
                        ysb = yout.tile([128, 256], F32, tag="y")
                        nc.scalar.copy(ysb[:], yp[:, 0:256])
                        eng_t["act"] += 400
                        nc.sync.dma_start(
                            y[q0 + m * 128:q0 + (m + 1) * 128, :], ysb[:])
                    pending.append(outp)
            flush_pending(len(pending))

    nc.compile()
    return nc


def _pv_dn(nc, o_ps, dn_ps, v_sb, ones_sb, started, last_ev, ex, kt, hp, di,
           ei):
    # PSUM "start" marks the whole 2KB zero region pending-zero, so exactly
    # one matmul per PSUM tile may carry start=True; every other group's
    # first write is auto-zeroed by that region-wide pending state.
    c0 = di if di > 0 else 0
    ei_end = max(last_ev.values())
    for hl in range(2):
        h = 2 * hp + hl
        for c in range(c0, 4):
            stop = ei == ei_end
            lhsT = ex[:, QT * hl + c * 128: QT * hl + (c + 1) * 128]
            nc.tensor.matmul(o_ps[:, c, 32 * h:32 * h + 32],
                             lhsT, v_sb[:, kt, 32 * h:32 * h + 32],
                             start="o" not in started, stop=stop,
                             skip_group_check=True)
            started.add("o")
            nc.tensor.matmul(dn_ps[:, c, h:h + 1],
                             lhsT, ones_sb[:],
                             start="dn" not in started, stop=stop,
                             skip_group_check=True)
            started.add("dn")


def _bf(x):
    return np.ascontiguousarray(np.asarray(x, np.float32)).astype(ml_dtypes.bfloat16)


def _host_prep(inputs, T):
    content = np.asarray(inputs["content"], np.float32)
    category = np.asarray(inputs["category"], np.float32)
    Wqc = np.asarray(inputs["Wqc"], np.float32)
    Wkc = np.asarray(inputs["Wkc"], np.float32)
    Wv = np.asarray(inputs["Wv"], np.float32)
    Wqk = np.asarray(inputs["Wqk"], np.float32)
    Wkk = np.asarray(inputs["Wkk"], np.float32)
    Wo = np.asarray(inputs["Wo"], np.float32)
    alpha = 1.0 / (1.0 + np.exp(-float(np.asarray(inputs["alpha_logit"]))))
    nb = content.shape[0]

    scale_q = (HD ** -0.5) * LOG2E / KEXP

    def wchunks(W, s, scale=1.0):
        wt = (W.T * scale)[:, 128 * s:128 * (s + 1)]
        return _bf(wt.reshape(2, 128, 128))

    p_idx = np.arange(128)[:, None]
    qcol = np.arange(KT)[None, :]
    mzro = np.tile((qcol >= p_idx).astype(np.float32), (1, 2))
    mzro = _bf(mzro)
    ident = _bf(np.eye(128, dtype=np.float32))

    in_maps = []
    for core in range(2 * nb):
        b, s = core // 2, core % 2
        m = {
            "xc": _bf(content[b].T.reshape(2, 128, T)),
            "xk": _bf(category[b].T.reshape(2, 128, T)),
            "wqc": wchunks(Wqc, s, scale_q),
            "wkc": wchunks(Wkc, s),
            "wqk": wchunks(Wqk, s, scale_q),
            "wkk": wchunks(Wkk, s),
            "wv": wchunks(Wv, s),
            "woc": _bf(Wo.T[128 * s:128 * (s + 1), :] * (1.0 - alpha)),
            "wok": _bf(Wo.T[128 * s:128 * (s + 1), :] * alpha),
            "ident": ident,
            "mzro": mzro,
        }
        in_maps.append(m)
    return in_maps


def _check_mask(mask, T):
    exp = np.triu(np.ones((T, T), dtype=bool), k=1)
    return np.array_equal(np.asarray(mask), exp)


def run(inputs, T=2048, cores=None, causal=True, **run_kwargs):
    """Build/compile (cached), run on hardware, return BassKernelResults."""
    key = (T, causal)
    if key not in _prog_cache:
        _prog_cache[key] = _build_program(T, causal=causal)
    nc = _prog_cache[key]
    in_maps = _host_prep(inputs, T)
    if cores is None:
        cores = list(range(len(in_maps)))
    res = run_bass_kernel_spmd(nc, [in_maps[c] for c in cores],
                               core_ids=list(range(len(cores))), **run_kwargs)
    return res


def kernel(**inputs):
    T = 2048
    mask = np.asarray(inputs["causal_mask"])
    if _check_mask(mask, T):
        causal = True
    elif not mask.any():
        causal = False
    else:
        raise NotImplementedError("kernel supports causal or empty masks only")
    res = run(inputs, T=T, causal=causal)
    nb = np.asarray(inputs["content"]).shape[0]
    bo = np.asarray(inputs["bo"], np.float32)
    out = np.empty((nb, T, D), np.float32)
    for b in range(nb):
        out[b] = res.results[2 * b]["y"] + res.results[2 * b + 1]["y"] + bo
    return out


# revision 12
# speedup vs baseline: 1.3320x; 1.0906x over previous
"""DIF (dual-softmax) attention layer on 8 Trainium2 NeuronCores.

Sharding: core = (batch b, head-stack s), b in 0..3, s in 0..1.
Each core computes, for its batch and its 4 heads, the full dual-softmax
attention over all T rows, producing a partial output projection (sum over
its 4 heads). Host sums the two stack partials and adds the bias.

On-chip layout, q-tile=512, k-tile=128:
  - Q^T, K^T per branch: [128 (4h x 32d), T] bf16 in SBUF; attention scale
    * log2(e)/KEXP folded into the Q projection weights (exp runs in base 2).
  - scores (S^T layout): per (k-tile, head-pair) event, 2 row-packed matmuls
    (tile_position=(32h,0)) into one [128 k, 1024 (2h x 512q)] PSUM tile.
  - exp: split between ACT (activation Exp, scale=KEXP*ln2) and DVE
    (EXP2P poly + EXP2SQ5 squarings custom ops), greedily balanced at build
    time.  Causal triangles of diagonal k-tiles are zeroed post-exp by the
    Pool engine (mzro multiply, SBUF-only).
  - P@V: per (kt, head, q-chunk of 128): out O[128 q, 32 d] full-partition
    matmul (lhsT = exp-scores chunk, rhs = V tile), accumulating over kt in
    PSUM; denominator = same lhsT vs a ones column -> dn[128 q, 1].
    This makes PV+denom ~8x cheaper than col-packed O^T accumulation.
  - normalize: reciprocal(dn) broadcast-multiplied into O (stride-0 AP),
    PSUM -> SBUF bf16.
  - O^T for the output projection via PE transpose ([128,128] bf16 blocks)
    + DVE copy back to SBUF.
  - output projection: lhsT = O^T chunks, rhs = Wo^T slices pre-scaled by
    alpha / (1-alpha); both branches accumulate into one PSUM bank; copied
    out and DMA'd; bias is added on the host.
"""

import numpy as np
import ml_dtypes

import concourse.bass as bass
import concourse.tile as tile
from concourse import bacc, mybir, dve_ops
from concourse.dve_spec import (Spec, Src0, C0, C1, C2, C3, One, Idx,
                                lower, _spill_c3_to_src1, _has_src1 as has_src1)
from concourse.dve_uop import DveOpSpec
from concourse.bass_utils import run_bass_kernel_spmd

B, D, H, HD = 4, 256, 8, 32
HPS = 4  # heads per stack (per core)
LOG2E = 1.4426950408889634
LN2 = 0.6931471805599453
QT = 512  # q-tile width
KT = 128  # k-tile width
KEXP = 32  # exp2 split factor: exp2(y) = p(y/KEXP)^KEXP on the DVE path

# minimax coefficients for p(z) = 1 + z(a + z(b + z(c + d z))) ~ 2^z, |z|<=0.5
PA, PB, PC, PD = 0.693128038, 0.24023678, 0.055870371, 0.009590248

BF16 = mybir.dt.bfloat16
F32 = mybir.dt.float32
AF = mybir.ActivationFunctionType
OP = mybir.AluOpType

_prog_cache: dict = {}


def _register_dve_op(name, spec, subdim=False):
    """Register a custom DVE op at import time, self-pinning its uops sha."""
    for op in dve_ops.OPS:
        if op.name == name:
            return op
    row = dve_ops._CUSTOM_DVE_ROW_BASE + len(dve_ops.OPS)
    shas = {}
    for ver in ("v3", "v4"):
        s = DveOpSpec(name=name, opcode=row, uops=lower(spec, ver=ver),
                      rd1_en=has_src1(spec))
        shas[ver] = s.sha(ver)
    op = dve_ops.DveOp(name, spec, subdim=subdim, uops_sha=shas)
    dve_ops.OPS.append(op)
    dve_ops._SUB_OPCODE_FOR_NAME[name] = row
    dve_ops.CUSTOM_DVE_SPECS[name] = spec
    return op


def _make_exp_ops():
    z = Src0
    poly = One + z * (C0 + z * (C1 + z * (C2 + C3 * z)))
    k1 = _register_dve_op("ANT_EXP2_POLY", Spec(
        body=_spill_c3_to_src1(poly),
        reference=lambda in0, in1, s0, s1, imm2:
            1.0 + in0 * (s0 + in0 * (s1 + in0 * (
                imm2 + np.reshape(in1, (-1,) + (1,) * (in0.ndim - 1)) * in0))),
    ))
    p = Src0
    for _ in range(5):
        p = p * p
    k2 = _register_dve_op("ANT_EXP2_SQ5", Spec(
        body=p,
        reference=lambda in0, in1, s0, s1, imm2: in0.astype(np.float32) ** 32,
    ))
    return k1, k2


EXP2P, EXP2SQ5 = _make_exp_ops()

# engine-balance cost constants (ns), from the TRN2 instruction cost model
_ACT_COL = 0.8333
_ACT_OVH = 190.0
_DVE_COL = 2.0833   # two custom-op passes
_DVE_OVH = 250.0


def _build_program(T, causal=True):
    nc = bacc.Bacc("TRN2", target_bir_lowering=False, debug=False)

    xc = nc.dram_tensor("xc", [2, 128, T], BF16, kind="ExternalInput")
    xk = nc.dram_tensor("xk", [2, 128, T], BF16, kind="ExternalInput")
    wqc = nc.dram_tensor("wqc", [2, 128, 128], BF16, kind="ExternalInput")
    wkc = nc.dram_tensor("wkc", [2, 128, 128], BF16, kind="ExternalInput")
    wqk = nc.dram_tensor("wqk", [2, 128, 128], BF16, kind="ExternalInput")
    wkk = nc.dram_tensor("wkk", [2, 128, 128], BF16, kind="ExternalInput")
    wv = nc.dram_tensor("wv", [2, 128, 128], BF16, kind="ExternalInput")
    woc = nc.dram_tensor("woc", [128, 256], BF16, kind="ExternalInput")
    wok = nc.dram_tensor("wok", [128, 256], BF16, kind="ExternalInput")
    ident = nc.dram_tensor("ident", [128, 128], BF16, kind="ExternalInput")
    mzro = nc.dram_tensor("mzro", [128, 2 * KT], BF16, kind="ExternalInput")
    y = nc.dram_tensor("y", [T, 256], F32, kind="ExternalOutput")

    NQT = T // QT
    NTT = T // KT
    NCH = QT // 128  # q-chunks per q-tile

    # build-time greedy engine balance for the exp events
    eng_t = {"act": 0.0, "dve": 0.0}

    import os
    force = os.environ.get("EXP_ENGINE", "")

    def pick_exp_engine(cols):
        if force:
            return force
        fa = eng_t["act"] + cols * _ACT_COL + _ACT_OVH
        fd = eng_t["dve"] + cols * _DVE_COL + _DVE_OVH
        if fa <= fd:
            eng_t["act"] = fa
            return "act"
        eng_t["dve"] = fd
        return "dve"

    with tile.TileContext(nc) as tc:
        with (
            tc.tile_pool(name="xin", bufs=1) as xin,
            tc.tile_pool(name="wts", bufs=1) as wts,
            tc.tile_pool(name="proj", bufs=1) as proj,
            tc.tile_pool(name="exps", bufs=4) as exps,
            tc.tile_pool(name="ex1p", bufs=3) as ex1p,
            tc.tile_pool(name="onrm", bufs=2) as onrm,
            tc.tile_pool(name="otsb", bufs=10) as otsb,
            tc.tile_pool(name="recp", bufs=2) as recp,
            tc.tile_pool(name="yout", bufs=3) as yout,
            tc.tile_pool(name="ps_sc", bufs=3, space="PSUM") as ps_sc,
            tc.tile_pool(name="ps_o", bufs=1, space="PSUM") as ps_o,
            tc.tile_pool(name="ps_dn", bufs=1, space="PSUM") as ps_dn,
        ):
            # ---- load inputs ----
            xc0 = xin.tile([128, T], BF16, tag="xc0")
            xc1 = xin.tile([128, T], BF16, tag="xc1")
            xk0 = xin.tile([128, T], BF16, tag="xk0")
            xk1 = xin.tile([128, T], BF16, tag="xk1")
            nch = 2 if T >= 1024 else 1
            for ch in range(nch):
                sl = bass.ts(ch, T // nch)
                nc.sync.dma_start(xc0[:, sl], xc[0][:, sl])
                nc.sync.dma_start(xc1[:, sl], xc[1][:, sl])
                nc.sync.dma_start(xk0[:, sl], xk[0][:, sl])
                nc.sync.dma_start(xk1[:, sl], xk[1][:, sl])

            # warm the ACT exp table while DMAs stream in
            warm = wts.tile([128, 1], F32, tag="warm")
            nc.vector.memset(warm[:], 0.0)
            nc.scalar.activation(warm[:], warm[:], AF.Exp, scale=1.0)

            w_sb = {}
            for nm, dram in [("wqc", wqc), ("wkc", wkc), ("wqk", wqk),
                             ("wkk", wkk), ("wv", wv)]:
                for j in range(2):
                    t = wts.tile([128, 128], BF16, tag=f"{nm}{j}")
                    nc.sync.dma_start(t[:], dram[j])
                    w_sb[(nm, j)] = t
            woc_sb = wts.tile([128, 256], BF16, tag="woc")
            wok_sb = wts.tile([128, 256], BF16, tag="wok")
            id_sb = wts.tile([128, 128], BF16, tag="ident")
            mzro_sb = wts.tile([128, 2 * KT], BF16, tag="mzro")
            nc.sync.dma_start(woc_sb[:], woc[:])
            nc.sync.dma_start(wok_sb[:], wok[:])
            nc.sync.dma_start(id_sb[:], ident[:])
            nc.sync.dma_start(mzro_sb[:], mzro[:])
            ones_sb = wts.tile([128, 1], BF16, tag="ones")
            nc.vector.memset(ones_sb[:], 1.0)
            dco_sb = wts.tile([128, 1], F32, tag="dco")
            nc.vector.memset(dco_sb[:], PD)

            # ---- projections ----
            # Q^T/K^T: out[i, t] = sum_j W.T[j, i] * x^T[j, t]
            qkt = {}
            for pi, (nm, xs) in enumerate([
                    ("wqc", (xc0, xc1)), ("wkc", (xc0, xc1)),
                    ("wqk", (xk0, xk1)), ("wkk", (xk0, xk1))]):
                dst = proj.tile([128, T], BF16, tag=f"p_{nm}")
                qkt[nm] = dst
                for nt in range(T // 512):
                    ps = ps_sc.tile([128, 1024], F32, tag="sc")
                    sl = bass.ts(nt, 512)
                    nc.tensor.matmul(ps[:, 0:512], w_sb[(nm, 0)][:],
                                     xs[0][:, sl], start=True, stop=False)
                    nc.tensor.matmul(ps[:, 0:512], w_sb[(nm, 1)][:],
                                     xs[1][:, sl], start=False, stop=True)
                    if (pi * (T // 512) + nt) % 2 == 0:
                        nc.vector.tensor_copy(dst[:, sl], ps[:, 0:512])
                    else:
                        nc.scalar.copy(dst[:, sl], ps[:, 0:512])
            q_c, k_c, q_k, k_k = qkt["wqc"], qkt["wkc"], qkt["wqk"], qkt["wkk"]

            # V: out[t, i] = sum_j x^T[j, t] * Wv.T[j, i]; layout [128, NTT, 128]
            v_sb = proj.tile([128, NTT, 128], BF16, tag="p_v")
            for tt in range(NTT):
                ps = ps_sc.tile([128, 1024], F32, tag="sc")
                sl = bass.ts(tt, 128)
                nc.tensor.matmul(ps[:, 0:128], xc0[:, sl], w_sb[("wv", 0)][:],
                                 start=True, stop=False)
                nc.tensor.matmul(ps[:, 0:128], xc1[:, sl], w_sb[("wv", 1)][:],
                                 start=False, stop=True)
                if tt % 2 == 0:
                    nc.scalar.copy(v_sb[:, tt, :], ps[:, 0:128])
                else:
                    nc.vector.tensor_copy(v_sb[:, tt, :], ps[:, 0:128])

            # ---- attention ----
            # deferred PE ops (transposes / out-proj) flushed between events
            pending = []

            def flush_pending(n=1):
                for _ in range(min(n, len(pending))):
                    pending.pop(0)()

            for qt in range(NQT):
                q0 = qt * QT
                nkt = (q0 + QT) // KT if causal else NTT
                nmask = 4 if causal else 0
                kts_clean = list(range(nkt - nmask))
                kts_mask = list(range(nkt - nmask, nkt))
                # interleave diagonal k-tiles among the clean ones
                order = []
                if kts_clean:
                    stride = max(1, len(kts_clean) // 4)
                    mi = 0
                    for i, kt in enumerate(kts_clean):
                        order.append(kt)
                        if (i + 1) % stride == 0 and mi < nmask:
                            order.append(kts_mask[mi])
                            mi += 1
                    order += kts_mask[mi:]
                else:
                    order = kts_mask

                ot_tiles = {}  # (br, m) -> O^T sbuf chunk
                for br, (qsb, ksb) in [("c", (q_c, k_c)), ("k", (q_k, k_k))]:
                    o_ps = ps_o.tile([128, NCH, 128], F32, tag="o")
                    dn_ps = ps_dn.tile([128, NCH, HPS], F32, tag="dn")
                    started = set()
                    # last event index per chunk (for stop flags)
                    last_ev = {}
                    for ei, kt in enumerate(order):
                        di = kt - (nkt - 4) if causal else -1
                        c0 = di if di > 0 else 0
                        for c in range(c0, NCH):
                            last_ev[c] = ei
                    pend = []
                    for ei, kt in enumerate(order):
                        k0 = kt * KT
                        di = kt - (nkt - 4) if causal else -1
                        qlo = 128 * di if di > 0 else 0
                        for hp in range(2):
                            sp = ps_sc.tile([128, 2 * QT], F32, tag="sc")
                            for hl in range(2):
                                h = 2 * hp + hl
                                nc.tensor.matmul(
                                    sp[:, QT * hl + qlo: QT * (hl + 1)],
                                    ksb[32 * h:32 * h + 32, k0:k0 + KT],
                                    qsb[32 * h:32 * h + 32, q0 + qlo:q0 + QT],
                                    start=True, stop=True,
                                    tile_position=(32 * h, 0),
                                    skip_group_check=True,
                                )
                            ex = exps.tile([128, 2 * QT], BF16, tag="ex")
                            cols = 2 * (QT - qlo)
                            eng = pick_exp_engine(cols)
                            if eng == "dve":
                                e1 = ex1p.tile([128, 2 * QT], F32, tag="e1")
                                if qlo:
                                    spv = sp[:].rearrange(
                                        "p (l q) -> p l q", l=2)[:, :, qlo:]
                                    e1v = e1[:].rearrange(
                                        "p (l q) -> p l q", l=2)[:, :, qlo:]
                                    exv = ex[:].rearrange(
                                        "p (l q) -> p l q", l=2)[:, :, qlo:]
                                else:
                                    spv, e1v, exv = sp[:], e1[:], ex[:]
                                nc.vector._custom_dve(
                                    EXP2P, out=e1v, in0=spv,
                                    in1=dco_sb[:, 0:1], s0=PA, s1=PB, imm2=PC)
                                nc.vector._custom_dve(EXP2SQ5, out=exv, in0=e1v)
                            else:
                                if qlo:
                                    nc.scalar.activation(
                                        ex[:].rearrange("p (l q) -> p l q", l=2)[:, :, qlo:],
                                        sp[:].rearrange("p (l q) -> p l q", l=2)[:, :, qlo:],
                                        AF.Exp, scale=KEXP * LN2)
                                else:
                                    nc.scalar.activation(ex[:], sp[:], AF.Exp,
                                                         scale=KEXP * LN2)
                            if di >= 0:
                                # zero the causal triangle post-exp (Pool)
                                exv = ex[:].rearrange(
                                    "p (l q) -> p l q", l=2)[:, :, qlo:qlo + KT]
                                nc.gpsimd.tensor_tensor(
                                    exv, exv,
                                    mzro_sb[:].rearrange("p (l q) -> p l q", l=2),
                                    OP.mult)
                            if len(pend) == 2:
                                _pv_dn(nc, o_ps, dn_ps, v_sb, ones_sb,
                                       started, last_ev, *pend.pop(0))
                            flush_pending(1)
                            pend.append((ex, kt, hp, di, ei))
                    for pe in pend:
                        _pv_dn(nc, o_ps, dn_ps, v_sb, ones_sb, started,
                               last_ev, *pe)

                    # normalize + transpose (deferred PE work via pending)
                    def norm_and_tp(br=br, o_ps=o_ps, dn_ps=dn_ps,
                                    ot_tiles=ot_tiles):
                        rec = recp.tile([128, NCH, HPS], F32, tag="rec")
                        nc.vector.reciprocal_approx_fast(
                            rec[:].rearrange("p c h -> p (c h)"),
                            dn_ps[:].rearrange("p c h -> p (c h)"))
                        eng_t["dve"] += 140
                        on = onrm.tile([128, NCH, 128], BF16, tag=f"on{br}")
                        rec_bc = rec[:].unsqueeze(3).broadcast_to(
                            [128, NCH, HPS, 32])
                        nc.vector.tensor_tensor(
                            on[:].rearrange("p c (h d) -> p c h d", h=HPS),
                            o_ps[:].rearrange("p c (h d) -> p c h d", h=HPS),
                            rec_bc, OP.mult)
                        eng_t["dve"] += 660
                        for m in range(NCH):
                            def tp(m=m, on=on, br=br, ot_tiles=ot_tiles):
                                ot_ps = ps_sc.tile([128, 2048], BF16, tag="sc")
                                nc.tensor.matmul(ot_ps[:, 0:128], on[:, m, :],
                                                 id_sb[:], is_transpose=True,
                                                 start=True, stop=True)
                                ot = otsb.tile([128, 128], BF16, tag="ot")
                                nc.vector.tensor_copy(ot[:], ot_ps[:, 0:128])
                                eng_t["dve"] += 200
                                ot_tiles[(br, m)] = ot
                            pending.append(tp)
                    norm_and_tp()

                # out-projection, deferred between next events
                for m in range(NCH):
                    def outp(m=m, qt=qt, q0=q0, ot_tiles=ot_tiles):
                        yp = ps_sc.tile([128, 1024], F32, tag="sc")
                        nc.tensor.matmul(yp[:, 0:256], ot_tiles[("c", m)][:],
                                         woc_sb[:], start=True, stop=False)
                        nc.tensor.matmul(yp[:, 0:256], ot_tiles[("k", m)][:],
                                         wok_sb[:], start=False, stop=True)
                        ysb = yout.tile([128, 256], F32, tag="y")
                        nc.scalar.copy(ysb[:], yp[:, 0:256])
                        eng_t["act"] += 400
                        nc.sync.dma_start(
                            y[q0 + m * 128:q0 + (m + 1) * 128, :], ysb[:])
                    pending.append(outp)
            flush_pending(len(pending))

    nc.compile()
    return nc


def _pv_dn(nc, o_ps, dn_ps, v_sb, ones_sb, started, last_ev, ex, kt, hp, di,
           ei):
    # PSUM "start" marks the whole 2KB zero region pending-zero, so exactly
    # one matmul per PSUM tile may carry start=True; every other group's
    # first write is auto-zeroed by that region-wide pending state.
    c0 = di if di > 0 else 0
    ei_end = max(last_ev.values())
    for hl in range(2):
        h = 2 * hp + hl
        for c in range(c0, 4):
            stop = ei == ei_end
            lhsT = ex[:, QT * hl + c * 128: QT * hl + (c + 1) * 128]
            nc.tensor.matmul(o_ps[:, c, 32 * h:32 * h + 32],
                             lhsT, v_sb[:, kt, 32 * h:32 * h + 32],
                             start="o" not in started, stop=stop,
                             skip_group_check=True)
            started.add("o")
            nc.tensor.matmul(dn_ps[:, c, h:h + 1],
                             lhsT, ones_sb[:],
                             start="dn" not in started, stop=stop,
                             skip_group_check=True)
            started.add("dn")


def _bf(x):
    return np.ascontiguousarray(np.asarray(x, np.float32)).astype(ml_dtypes.bfloat16)


def _host_prep(inputs, T):
    content = np.asarray(inputs["content"], np.float32)
    category = np.asarray(inputs["category"], np.float32)
    Wqc = np.asarray(inputs["Wqc"], np.float32)
    Wkc = np.asarray(inputs["Wkc"], np.float32)
    Wv = np.asarray(inputs["Wv"], np.float32)
    Wqk = np.asarray(inputs["Wqk"], np.float32)
    Wkk = np.asarray(inputs["Wkk"], np.float32)
    Wo = np.asarray(inputs["Wo"], np.float32)
    alpha = 1.0 / (1.0 + np.exp(-float(np.asarray(inputs["alpha_logit"]))))
    nb = content.shape[0]

    scale_q = (HD ** -0.5) * LOG2E / KEXP

    def wchunks(W, s, scale=1.0):
        wt = (W.T * scale)[:, 128 * s:128 * (s + 1)]
        return _bf(wt.reshape(2, 128, 128))

    p_idx = np.arange(128)[:, None]
    qcol = np.arange(KT)[None, :]
    mzro = np.tile((qcol >= p_idx).astype(np.float32), (1, 2))
    mzro = _bf(mzro)
    ident = _bf(np.eye(128, dtype=np.float32))

    in_maps = []
    for core in range(2 * nb):
        b, s = core // 2, core % 2
        m = {
            "xc": _bf(content[b].T.reshape(2, 128, T)),
            "xk": _bf(category[b].T.reshape(2, 128, T)),
            "wqc": wchunks(Wqc, s, scale_q),
            "wkc": wchunks(Wkc, s),
            "wqk": wchunks(Wqk, s, scale_q),
            "wkk": wchunks(Wkk, s),
            "wv": wchunks(Wv, s),
            "woc": _bf(Wo.T[128 * s:128 * (s + 1), :] * (1.0 - alpha)),
            "wok": _bf(Wo.T[128 * s:128 * (s + 1), :] * alpha),
            "ident": ident,
            "mzro": mzro,
        }
        in_maps.append(m)
    return in_maps


def _check_mask(mask, T):
    exp = np.triu(np.ones((T, T), dtype=bool), k=1)
    return np.array_equal(np.asarray(mask), exp)


def run(inputs, T=2048, cores=None, causal=True, **run_kwargs):
    """Build/compile (cached), run on hardware, return BassKernelResults."""
    key = (T, causal)
    if key not in _prog_cache:
        _prog_cache[key] = _build_program(T, causal=causal)
    nc = _prog_cache[key]
    in_maps = _host_prep(inputs, T)
    if cores is None:
        cores = list(range(len(in_maps)))
    res = run_bass_kernel_spmd(nc, [in_maps[c] for c in cores],
                               core_ids=list(range(len(cores))), **run_kwargs)
    return res


def kernel(**inputs):
    T = 2048
    mask = np.asarray(inputs["causal_mask"])
    if _check_mask(mask, T):
        causal = True
    elif not mask.any():
        causal = False
    else:
        raise NotImplementedError("kernel supports causal or empty masks only")
    res = run(inputs, T=T, causal=causal)
    nb = np.asarray(inputs["content"]).shape[0]
    bo = np.asarray(inputs["bo"], np.float32)
    out = np.empty((nb, T, D), np.float32)
    for b in range(nb):
        out[b] = res.results[2 * b]["y"] + res.results[2 * b + 1]["y"] + bo
    return out
